# revision 1
# baseline (speedup 1.0000x reference)
"""GCN (2-layer GraphConv + classifier) on 8 Trainium2 NeuronCores.

Strategy: shard nodes (and their incident edges, by dst) across the 8 cores;
replicate the small weight matrices; AllGather the per-layer node features so
every core can gather its edges' source rows; segment-sum via one-hot
M-matmuls on the tensor engine, with edge rows fetched by SWDGE dma_gather.
The classifier weight is folded into layer 2 (W2 @ Wc), so layer 2 is 7-wide.
"""
import os
import sys

sys.path.insert(0, "/opt/trn_rl_repo")

import numpy as np
import ml_dtypes

import concourse.bacc as bacc
import concourse.bass as bass
import concourse.mybir as mybir
import concourse.tile as tile
from concourse import library_config
from concourse.masks import make_identity

NCORES = 8
P = 128
N_NODES = 50000
N_EDGES = 400000
NP_PAD = 50176            # 8 * 6272
R = NP_PAD // NCORES      # 6272 rows per core
RT = R // P               # 49 row tiles per core
HALF = NP_PAD // 2        # 25088 (< 32768 so int16 indices work per half)
IN_F = 1433
KP = 1536                 # padded contraction (12 * 128)
KC = KP // P              # 12 k-chunks
HID = 384
N_CLS = 7
ZC = 128                  # padded z row width (bf16 -> 256B rows for dma_gather)
GROUP_TILES = 4           # dst tiles per gather-call group

bf16 = ml_dtypes.bfloat16

_cache = {}


def _build_edge_plan(edge_src, edge_dst):
    """Partition edges by dst core; per (core, tile) split by src half; pad to
    128-edge chunks with a uniform (max-over-cores) per-(tile, half) chunk
    count so one NEFF fits all cores."""
    src = edge_src.astype(np.int64)
    dst = edge_dst.astype(np.int64)
    core = dst // R
    t_all = (dst % R) // P
    p_all = dst % P
    half_all = (src >= HALF).astype(np.int64)
    src_rel = src - HALF * half_all

    counts = np.zeros((NCORES, RT, 2), np.int64)
    np.add.at(counts, (core, t_all, half_all), 1)
    chunks = np.maximum(np.ceil(counts / P).astype(np.int64).max(axis=0), 1)
    chunks_lo = chunks[:, 0]
    chunks_hi = chunks[:, 1]

    n_groups = (RT + GROUP_TILES - 1) // GROUP_TILES
    groups = []
    chunk_base = 0
    lo_off = np.zeros(RT, np.int64)
    hi_off = np.zeros(RT, np.int64)
    for g in range(n_groups):
        tiles = list(range(g * GROUP_TILES, min((g + 1) * GROUP_TILES, RT)))
        lo_tot = int(chunks_lo[tiles].sum())
        hi_tot = int(chunks_hi[tiles].sum())
        ofs = chunk_base
        for t in tiles:
            lo_off[t] = ofs
            ofs += chunks_lo[t]
        for t in tiles:
            hi_off[t] = ofs
            ofs += chunks_hi[t]
        groups.append((tiles, lo_tot, hi_tot, chunk_base))
        chunk_base += lo_tot + hi_tot
    c_tot = chunk_base

    order = np.lexsort((src_rel, half_all, t_all, core))
    M_all = np.zeros((NCORES, c_tot, P, P), bf16)
    idx_all = np.zeros((NCORES, c_tot * P), np.int64)

    srt_core = core[order]
    srt_t = t_all[order]
    srt_half = half_all[order]
    srt_src = src_rel[order]
    srt_p = p_all[order]

    core_starts = np.searchsorted(srt_core, np.arange(NCORES + 1))
    for c in range(NCORES):
        s, e = core_starts[c], core_starts[c + 1]
        tt = srt_t[s:e]
        hh = srt_half[s:e]
        ss = srt_src[s:e]
        pp = srt_p[s:e]
        key = tt * 2 + hh
        if len(key):
            new_run = np.concatenate([[True], key[1:] != key[:-1]])
            run_ids = np.cumsum(new_run) - 1
            first_pos = np.nonzero(new_run)[0]
            run_start = first_pos[run_ids]
            pos_in_run = np.arange(len(key)) - run_start
            base = np.where(hh == 0, lo_off[tt], hi_off[tt]) * P
            gpos = base + pos_in_run
            idx_all[c][gpos] = ss
            M_all[c][gpos // P, gpos % P, pp] = 1

    # M pre-swizzled for DMA: [P(edge k), c_tot*P(dst cols)]
    M_sw = np.ascontiguousarray(M_all.transpose(0, 2, 1, 3)).reshape(NCORES, P, c_tot * P)

    idx_wrapped = np.zeros((NCORES, P, c_tot * P // 16), np.int16)
    for c in range(NCORES):
        w = idx_all[c].astype(np.int16).reshape(-1, 16).T
        idx_wrapped[c] = np.tile(w, (8, 1))

    return dict(
        chunks_lo=chunks_lo, chunks_hi=chunks_hi, groups=groups, c_tot=c_tot,
        M_sw=M_sw, idx_wrapped=idx_wrapped,
    )


def _build_nc(plan, repeat=1):
    chunks_lo = plan["chunks_lo"]
    chunks_hi = plan["chunks_hi"]
    groups = plan["groups"]
    c_tot = plan["c_tot"]

    nc = bacc.Bacc("TRN2", target_bir_lowering=False, debug=False,
                   num_devices=NCORES)
    dt = mybir.dt

    # ---- I/O ----
    xT = nc.dram_tensor("xT", [RT, P, KC * P], dt.bfloat16, kind="ExternalInput")
    w1 = nc.dram_tensor("w1", [P, KC * HID], dt.bfloat16, kind="ExternalInput")
    w2c = nc.dram_tensor("w2c", [P, 3 * 8], dt.bfloat16, kind="ExternalInput")
    b1t = nc.dram_tensor("b1t", [P, HID], dt.float32, kind="ExternalInput")
    bct = nc.dram_tensor("bct", [P, 8], dt.float32, kind="ExternalInput")
    inv_s_t = nc.dram_tensor("inv_s_t", [P, RT], dt.float32, kind="ExternalInput")
    inv_d_t = nc.dram_tensor("inv_d_t", [P, RT], dt.float32, kind="ExternalInput")
    m_all = nc.dram_tensor("m_all", [P, c_tot * P], dt.bfloat16, kind="ExternalInput")
    idxs = nc.dram_tensor("idxs", [P, c_tot * P // 16], dt.int16, kind="ExternalInput")
    out = nc.dram_tensor("out", [P, RT * N_CLS], dt.float32, kind="ExternalOutput")

    # ---- internal DRAM ----
    h_c = nc.dram_tensor("h_c", [R, HID], dt.bfloat16)
    h_full = nc.dram_tensor("h_full", [NP_PAD, HID], dt.bfloat16, addr_space="Shared")
    z_c = nc.dram_tensor("z_c", [R, 8], dt.bfloat16)
    z_fulln = nc.dram_tensor("z_fulln", [NP_PAD, 8], dt.bfloat16, addr_space="Shared")
    z_full = nc.dram_tensor("z_full", [NP_PAD, ZC], dt.bfloat16)

    rg = [list(range(NCORES))]

    with tile.TileContext(nc) as tc:
        with (
            tc.tile_pool(name="const", bufs=1) as const,
            tc.tile_pool(name="xload", bufs=3) as xload,
            tc.tile_pool(name="hout", bufs=3) as hout,
            tc.tile_pool(name="glo", bufs=2) as glo,
            tc.tile_pool(name="ghi", bufs=2) as ghi,
            tc.tile_pool(name="mbuf", bufs=2) as mbuf,
            tc.tile_pool(name="work", bufs=4) as work,
            tc.tile_pool(name="h1Tp", bufs=1) as h1Tp,
            tc.tile_pool(name="psA", bufs=2, space="PSUM") as psA,
            tc.tile_pool(name="psB", bufs=2, space="PSUM") as psB,
        ):
            nc.gpsimd.load_library(library_config.mlp)

            w1_t = const.tile([P, KC * HID], dt.bfloat16)
            nc.sync.dma_start(out=w1_t[:], in_=w1[:])
            w2c_t = const.tile([P, 3 * 8], dt.bfloat16)
            nc.sync.dma_start(out=w2c_t[:], in_=w2c[:])
            b1_t = const.tile([P, HID], dt.float32)
            nc.sync.dma_start(out=b1_t[:], in_=b1t[:])
            bc_t = const.tile([P, 8], dt.float32)
            nc.sync.dma_start(out=bc_t[:], in_=bct[:])
            invs_t = const.tile([P, RT], dt.float32)
            nc.sync.dma_start(out=invs_t[:], in_=inv_s_t[:])
            invd_t = const.tile([P, RT], dt.float32)
            nc.sync.dma_start(out=invd_t[:], in_=inv_d_t[:])
            idx_t = const.tile([P, c_tot * P // 16], dt.int16)
            nc.sync.dma_start(out=idx_t[:], in_=idxs[:])
            ident = const.tile([P, P], dt.bfloat16)
            make_identity(nc, ident[:])

            h1T_t = h1Tp.tile([P, 3 * R], dt.bfloat16)  # [k=128][kchunk][row]

            for _rep in range(repeat):
                # ---- Phase 1: h = (x @ W1) * inv_s ----
                for r in range(RT):
                    xt = xload.tile([P, KC * P], dt.bfloat16)
                    nc.sync.dma_start(out=xt[:], in_=xT[r])
                    ps = psA.tile([P, HID], dt.float32, space="PSUM")
                    for k in range(KC):
                        nc.tensor.matmul(
                            out=ps[:],
                            lhsT=xt[:, k * P:(k + 1) * P],
                            rhs=w1_t[:, k * HID:(k + 1) * HID],
                            start=(k == 0),
                            stop=(k == KC - 1),
                        )
                    ht = hout.tile([P, HID], dt.bfloat16)
                    nc.scalar.activation(
                        out=ht[:], in_=ps[:],
                        func=mybir.ActivationFunctionType.Copy,
                        scale=invs_t[:, r:r + 1],
                    )
                    nc.sync.dma_start(out=h_c[r * P:(r + 1) * P, :], in_=ht[:])

                # ---- Phase 2: AllGather h ----
                nc.gpsimd.collective_compute(
                    "AllGather", mybir.AluOpType.bypass, replica_groups=rg,
                    ins=[h_c[:]], outs=[h_full[:]],
                )

                # ---- Phase 3: aggregation 1 + h1 transpose + z ----
                h_lo = h_full[0:HALF, :]
                h_hi = h_full[HALF:NP_PAD, :]
                for (tiles, lo_tot, hi_tot, cbase) in groups:
                    nlo = lo_tot * P
                    nhi = hi_tot * P
                    glo_t = glo.tile([P, lo_tot, HID], dt.bfloat16, tag="g1lo")
                    nc.gpsimd.dma_gather(
                        out_ap=glo_t[:], in_ap=h_lo,
                        idxs_ap=idx_t[:, cbase * 8:(cbase + lo_tot) * 8],
                        num_idxs=nlo, num_idxs_reg=nlo, elem_size=HID,
                        single_packet=False,
                    )
                    ghi_t = ghi.tile([P, hi_tot, HID], dt.bfloat16, tag="g1hi")
                    nc.gpsimd.dma_gather(
                        out_ap=ghi_t[:], in_ap=h_hi,
                        idxs_ap=idx_t[:, (cbase + lo_tot) * 8:(cbase + lo_tot + hi_tot) * 8],
                        num_idxs=nhi, num_idxs_reg=nhi, elem_size=HID,
                        single_packet=False,
                    )
                    m_t = mbuf.tile([P, (lo_tot + hi_tot) * P], dt.bfloat16, tag="m1")
                    nc.sync.dma_start(
                        out=m_t[:],
                        in_=m_all[:, cbase * P:(cbase + lo_tot + hi_tot) * P],
                    )
                    lo_pos = 0
                    hi_pos = 0
                    for t in tiles:
                        ncl = int(chunks_lo[t])
                        nch = int(chunks_hi[t])
                        ps = psA.tile([P, HID], dt.float32, space="PSUM", tag="ps")
                        for j in range(ncl):
                            mcol = (lo_pos + j) * P
                            nc.tensor.matmul(
                                out=ps[:], lhsT=m_t[:, mcol:mcol + P],
                                rhs=glo_t[:, lo_pos + j, :],
                                start=(j == 0), stop=False,
                            )
                        for j in range(nch):
                            mcol = (lo_tot + hi_pos + j) * P
                            nc.tensor.matmul(
                                out=ps[:], lhsT=m_t[:, mcol:mcol + P],
                                rhs=ghi_t[:, hi_pos + j, :],
                                start=False, stop=(j == nch - 1),
                            )
                        lo_pos += ncl
                        hi_pos += nch
                        # h1 = relu(agg * inv_d + b1)
                        tmp = work.tile([P, HID], dt.float32, tag="tmp1")
                        nc.vector.scalar_tensor_tensor(
                            out=tmp[:], in0=ps[:], scalar=invd_t[:, t:t + 1],
                            in1=b1_t[:],
                            op0=mybir.AluOpType.mult, op1=mybir.AluOpType.add,
                        )
                        h1t = work.tile([P, HID], dt.bfloat16, tag="h1t")
                        nc.vector.tensor_scalar_max(out=h1t[:], in0=tmp[:], scalar1=0.0)
                        for k in range(3):
                            pst = psB.tile([P, P], dt.bfloat16, space="PSUM", tag="pst")
                            nc.tensor.transpose(
                                out=pst[:], in_=h1t[:, k * P:(k + 1) * P],
                                identity=ident[:],
                            )
                            nc.vector.tensor_copy(
                                out=h1T_t[:, k * R + t * P: k * R + (t + 1) * P],
                                in_=pst[:],
                            )
                        psz = psB.tile([P, 8], dt.float32, space="PSUM", tag="psz")
                        for k in range(3):
                            nc.tensor.matmul(
                                out=psz[:],
                                lhsT=h1T_t[:, k * R + t * P: k * R + (t + 1) * P],
                                rhs=w2c_t[:, k * 8:(k + 1) * 8],
                                start=(k == 0), stop=(k == 2),
                            )
                        zt = work.tile([P, 8], dt.bfloat16, tag="zt")
                        nc.scalar.activation(
                            out=zt[:], in_=psz[:],
                            func=mybir.ActivationFunctionType.Copy,
                            scale=invs_t[:, t:t + 1],
                        )
                        nc.sync.dma_start(out=z_c[t * P:(t + 1) * P, :], in_=zt[:])

                # ---- Phase 4: AllGather narrow z, expand into padded table ----
                nc.gpsimd.collective_compute(
                    "AllGather", mybir.AluOpType.bypass, replica_groups=rg,
                    ins=[z_c[:]], outs=[z_fulln[:]],
                )
                nc.sync.dma_start(out=z_full[:, 0:8], in_=z_fulln[:])

                # ---- Phase 5: aggregation 2 (7-wide) ----
                z_lo = z_full[0:HALF, :]
                z_hi = z_full[HALF:NP_PAD, :]
                out_t = const.tile([P, RT * N_CLS], dt.float32)
                for (tiles, lo_tot, hi_tot, cbase) in groups:
                    nlo = lo_tot * P
                    nhi = hi_tot * P
                    glo_t = glo.tile([P, lo_tot, ZC], dt.bfloat16, tag="g2lo")
                    nc.gpsimd.dma_gather(
                        out_ap=glo_t[:], in_ap=z_lo,
                        idxs_ap=idx_t[:, cbase * 8:(cbase + lo_tot) * 8],
                        num_idxs=nlo, num_idxs_reg=nlo, elem_size=ZC,
                        single_packet=False,
                    )
                    ghi_t = ghi.tile([P, hi_tot, ZC], dt.bfloat16, tag="g2hi")
                    nc.gpsimd.dma_gather(
                        out_ap=ghi_t[:], in_ap=z_hi,
                        idxs_ap=idx_t[:, (cbase + lo_tot) * 8:(cbase + lo_tot + hi_tot) * 8],
                        num_idxs=nhi, num_idxs_reg=nhi, elem_size=ZC,
                        single_packet=False,
                    )
                    m_t = mbuf.tile([P, (lo_tot + hi_tot) * P], dt.bfloat16, tag="m2")
                    nc.sync.dma_start(
                        out=m_t[:],
                        in_=m_all[:, cbase * P:(cbase + lo_tot + hi_tot) * P],
                    )
                    lo_pos = 0
                    hi_pos = 0
                    for t in tiles:
                        ncl = int(chunks_lo[t])
                        nch = int(chunks_hi[t])
                        ps = psB.tile([P, 8], dt.float32, space="PSUM", tag="ps2")
                        for j in range(ncl):
                            mcol = (lo_pos + j) * P
                            nc.tensor.matmul(
                                out=ps[:], lhsT=m_t[:, mcol:mcol + P],
                                rhs=glo_t[:, lo_pos + j, 0:8],
                                start=(j == 0), stop=False,
                            )
                        for j in range(nch):
                            mcol = (lo_tot + hi_pos + j) * P
                            nc.tensor.matmul(
                                out=ps[:], lhsT=m_t[:, mcol:mcol + P],
                                rhs=ghi_t[:, hi_pos + j, 0:8],
                                start=False, stop=(j == nch - 1),
                            )
                        lo_pos += ncl
                        hi_pos += nch
                        nc.vector.scalar_tensor_tensor(
                            out=out_t[:, t * N_CLS:(t + 1) * N_CLS],
                            in0=ps[:, 0:N_CLS], scalar=invd_t[:, t:t + 1],
                            in1=bc_t[:, 0:N_CLS],
                            op0=mybir.AluOpType.mult, op1=mybir.AluOpType.add,
                        )
                nc.sync.dma_start(out=out[:], in_=out_t[:])

    nc.compile()
    return nc


def _prepare(features, edge_src, edge_dst, W1, b1, W2, b2, Wc, bc):
    deg_out = np.bincount(edge_src, minlength=N_NODES).astype(np.float32)
    deg_in = np.bincount(edge_dst, minlength=N_NODES).astype(np.float32)
    inv_s = 1.0 / np.sqrt(np.maximum(deg_out, 1.0))
    inv_d = 1.0 / np.sqrt(np.maximum(deg_in, 1.0))
    inv_s = np.concatenate([inv_s, np.ones(NP_PAD - N_NODES, np.float32)])
    inv_d = np.concatenate([inv_d, np.ones(NP_PAD - N_NODES, np.float32)])

    plan = _build_edge_plan(edge_src, edge_dst)

    W1p = np.zeros((KP, HID), np.float32)
    W1p[:IN_F] = W1
    W1p = W1p.astype(bf16)
    w1_sw = np.concatenate([W1p[k * P:(k + 1) * P] for k in range(KC)], axis=1)
    W2c = (W2.astype(np.float32) @ Wc.astype(np.float32))
    W2cp = np.zeros((HID, 8), np.float32)
    W2cp[:, :N_CLS] = W2c
    W2cp16 = W2cp.astype(bf16)
    w2c_sw = np.concatenate([W2cp16[k * P:(k + 1) * P] for k in range(3)], axis=1)
    bcp = (b2.astype(np.float32) @ Wc.astype(np.float32) + bc).astype(np.float32)
    b1_full = np.tile(b1[None, :].astype(np.float32), (P, 1))
    bc_full = np.zeros((P, 8), np.float32)
    bc_full[:, :N_CLS] = bcp[None, :]

    xpad = np.zeros((NP_PAD, KP), bf16)
    xpad[:N_NODES, :IN_F] = features.astype(bf16)

    in_maps = []
    for c in range(NCORES):
        xc = xpad[c * R:(c + 1) * R]
        # xt[r][kk, k*P+rr] = xc[r*P+rr, k*P+kk]
        xt = np.ascontiguousarray(
            xpad[c * R:(c + 1) * R].reshape(RT, P, KC, P).transpose(0, 3, 2, 1)
        ).reshape(RT, P, KC * P)
        inv_s_tile = np.ascontiguousarray(inv_s[c * R:(c + 1) * R].reshape(RT, P).T)
        inv_d_tile = np.ascontiguousarray(inv_d[c * R:(c + 1) * R].reshape(RT, P).T)
        in_maps.append({
            "xT": xt,
            "w1": w1_sw,
            "w2c": w2c_sw,
            "b1t": b1_full,
            "bct": bc_full,
            "inv_s_t": inv_s_tile,
            "inv_d_t": inv_d_tile,
            "m_all": plan["M_sw"][c],
            "idxs": plan["idx_wrapped"][c],
        })
    return plan, in_maps


def kernel(features, edge_src, edge_dst, W1, b1, W2, b2, Wc, bc):
    features = np.asarray(features, np.float32)
    edge_src = np.asarray(edge_src)
    edge_dst = np.asarray(edge_dst)
    plan, in_maps = _prepare(features, edge_src, edge_dst,
                             np.asarray(W1, np.float32), np.asarray(b1, np.float32),
                             np.asarray(W2, np.float32), np.asarray(b2, np.float32),
                             np.asarray(Wc, np.float32), np.asarray(bc, np.float32))
    nc = _build_nc(plan)

    from concourse.bass_utils import run_bass_kernel_spmd
    res = run_bass_kernel_spmd(nc, in_maps, core_ids=list(range(NCORES)))

    out = np.zeros((NP_PAD, N_CLS), np.float32)
    for c in range(NCORES):
        buf = res.results[c]["out"]
        out[c * R:(c + 1) * R] = buf.reshape(P, RT, N_CLS).transpose(1, 0, 2).reshape(R, N_CLS)
    return out[:N_NODES]

